# revision 39
# baseline (speedup 1.0000x reference)
"""Trainium2 Bass kernel for DeepMinAttLSTM (4x minLSTM + MHSA + last-step FC).

Strategy:
  - Data-parallel over batch: 16 batches -> 8 cores x 2 batches.
  - Everything on device is kept feature-major: activations live as
    X^T [H=1024 (8 partition-tiles of 128), B*S=2048 free] in bf16, plus an
    fp8 (x16) copy that feeds the fp8 matmuls.
  - Per layer: f,i gate matmuls in fp8 (DoubleRow perf mode: 2 contraction
    slices per pass = 2x flops), h~ matmul in bf16 (the value path needs the
    precision), fp32 PSUM everywhere.
  - Gate math is spread across four engines so the DVE (the scan owner)
    stays below the PE's pace:
      ACT:  d = sigmoid(psF + bf)  [fp32], i = sigmoid(psI + bi)
      DMA (gpsimd software DGE, CCE fp32 add):  d += i
      DVE:  r = 1/d (approx), w = i*r
      ACT:  fp = 1 - w  (affine: w*-1 + 1)
      DVE:  add = (psH + bh)*w, then one merged tensor_tensor_scan over the
            full S=1024 sequence of each batch (exact initial=0).
  - The final output only uses out[:, -1, :], so attention collapses to the
    last query position: K and V are computed for the full sequence (fp8
    DoubleRow), Q only for the last column per batch (fp8). The softmax-
    denominator chain is scheduled before the V matmuls so it hides under
    them. Softmax needs no max-subtraction (logit absmax ~ 0.01).
  - The attention/residual branch is ~2% of the output magnitude, so fp8
    K/V/Q error is negligible; fp8 f,i gate error is damped by the sigmoid
    slope and the f/(f+i) normalization (measured end-to-end ~1.3e-2).
"""

import math

import numpy as np
import ml_dtypes

BF16 = ml_dtypes.bfloat16
F8 = ml_dtypes.float8_e4m3      # == TRN FP8_EXP4 (max 240)

P = 128
H = 1024
S = 1024
B = 16
NCORES = 8
BC = B // NCORES          # batches per core
BS = BC * S               # 2048 free columns per core
KO = H // P               # 8 feature partition-tiles
NH = 8
DH = H // NH              # 128
O = 256
L = 4
QSCALE = 1.0 / math.sqrt(DH)

WS = 256.0                # fp8 weight scale
XS = 16.0                 # fp8 activation scale
RS = 1.0 / (WS * XS)      # PSUM descale for fp8 matmul results

_CACHE = {}


def _build_nc():
    import concourse.mybir as mybir
    import concourse.tile as tile
    from concourse import bacc

    DT = mybir.dt.bfloat16
    F8D = mybir.dt.float8e4
    F32 = mybir.dt.float32
    AFT = mybir.ActivationFunctionType
    OP = mybir.AluOpType
    DR = mybir.MatmulPerfMode.DoubleRow

    nc = bacc.Bacc("TRN2", target_bir_lowering=False, debug=False,
                   num_devices=NCORES)

    xT = nc.dram_tensor("xT", [P, KO * BS], DT, kind="ExternalInput").ap()
    # fp8 input, batch-major: columns (b, ko, s)
    xT8 = nc.dram_tensor("xT8", [P, BC * KO * S], F8D,
                         kind="ExternalInput").ap()
    gw8 = nc.dram_tensor("gw8", [2 * L * P, KO * H], F8D,
                         kind="ExternalInput").ap()
    gwh = nc.dram_tensor("gwh", [L * P, KO * H], DT, kind="ExternalInput").ap()
    gb = nc.dram_tensor("gb", [P, 3 * L * KO], F32, kind="ExternalInput").ap()
    ip8 = nc.dram_tensor("ip8", [P, KO * 3 * H], F8D,
                         kind="ExternalInput").ap()
    ipb = nc.dram_tensor("ipb", [P, 2 * KO], F32, kind="ExternalInput").ap()
    vb = nc.dram_tensor("vb", [P, NH], F32, kind="ExternalInput").ap()
    ow = nc.dram_tensor("ow", [P, KO * H], DT, kind="ExternalInput").ap()
    ob = nc.dram_tensor("ob", [P, KO], F32, kind="ExternalInput").ap()
    fcw = nc.dram_tensor("fcw", [P, KO * O], DT, kind="ExternalInput").ap()
    fcb = nc.dram_tensor("fcb", [P, O // P], F32, kind="ExternalInput").ap()
    outT = nc.dram_tensor("outT", [O, BC], F32, kind="ExternalOutput").ap()

    with tile.TileContext(nc) as tc:
        with (
            tc.tile_pool(name="constp", bufs=1) as constp,
            tc.tile_pool(name="hbuf", bufs=2) as hp,
            tc.tile_pool(name="h8buf", bufs=2) as h8p,
            tc.tile_pool(name="psA", bufs=6, space="PSUM") as psA,
        ):
            gb_sb = constp.tile([P, 3 * L * KO], F32)
            nc.sync.dma_start(gb_sb[:], gb[:])
            ones_col = constp.tile([P, 1], F32)
            nc.vector.memset(ones_col[:], 1.0)
            ones_row = constp.tile([1, P], F32)
            nc.vector.memset(ones_row[:], 1.0)

            X = hp.tile([P, KO * BS], DT, tag="hbuf", name="xT_sb")
            # fp8 activations live as one tile per batch so Tile's region
            # tracking isn't defeated by the rearranged 3D DoubleRow reads
            X8b = [h8p.tile([P, KO * S], F8D, tag=f"h8b{b}",
                            name=f"xT8_sb{b}") for b in range(BC)]
            # spread the startup-critical input loads across the DMA queues
            # (per-queue DMA throughput is the startup bottleneck)
            for b in range(BC):
                for kq in range(KO // 2):
                    c0, c1 = 2 * kq * S, (2 * kq + 2) * S
                    nc.sync.dma_start(X8b[b][:, c0:c1],
                                      xT8[:, b * KO * S + c0:
                                          b * KO * S + c1])
            for kq in range(KO // 2):
                c0, c1 = 2 * kq * BS, (2 * kq + 2) * BS
                nc.scalar.dma_start(X[:, c0:c1], xT[:, c0:c1])

            # in_proj weights preloaded early (pool below layer pools so the
            # DMA does not wait for layer-pool release zones)
            ip_pool = tc.tile_pool(name="ipp", bufs=1)
            ipp = ip_pool.__enter__()
            ip8_sb = ipp.tile([P, KO * 3 * H], F8D, name="ip8_sb")

            # ---------------- minLSTM layers ----------------
            with (
                tc.tile_pool(name="gw8p", bufs=4) as gw8p,
                tc.tile_pool(name="gwhp", bufs=2) as gwhp,
                tc.tile_pool(name="fpp", bufs=2) as fpp,
                tc.tile_pool(name="addp", bufs=2) as addp,
                tc.tile_pool(name="tmpp", bufs=2) as tmpp,
            ):
                for l in range(L):
                    if l == 2:
                        # overlap the in_proj load with layers 2-3
                        nc.sync.dma_start(ip8_sb[:], ip8[:])
                    gws8 = []
                    for g in range(2):
                        lg = l * 2 + g
                        g8_t = gw8p.tile([P, KO * H], F8D, tag="gw8",
                                         name=f"gw8_{l}_{g}")
                        # layer 0's gate weights ride the (startup-idle)
                        # gpsimd queue; afterwards gpsimd is reserved for the
                        # d+=i accum DMAs so they never wait behind weights
                        eng = nc.gpsimd if l == 0 else nc.sync
                        eng.dma_start(g8_t[:], gw8[lg * P:(lg + 1) * P, :])
                        gws8.append(g8_t.rearrange("p (k h) -> p k h", k=KO))
                    gwh_t = gwhp.tile([P, KO * H], DT, tag="gwh",
                                      name=f"gwh_{l}")
                    eng = nc.sync if l == 0 else nc.scalar
                    eng.dma_start(gwh_t[:], gwh[l * P:(l + 1) * P, :])

                    h_out = hp.tile([P, KO * BS], DT, tag="hbuf", name=f"h_{l}")
                    h8b_out = [h8p.tile([P, KO * S], F8D, tag=f"h8b{b}",
                                        name=f"h8_{l}_{b}")
                               for b in range(BC)]
                    X8b_v = [t.rearrange("p (k s) -> p k s", k=KO)
                             for t in X8b]
                    for b in range(BC):
                        for no in range(KO):
                            base = no * BS + b * S
                            fp_t = fpp.tile([P, S], DT, tag="fp",
                                            name=f"fp_{l}_{b}_{no}")
                            add_t = addp.tile([P, S], DT, tag="add",
                                              name=f"add_{l}_{b}_{no}")
                            # f,i fp8 DoubleRow matmuls for both sequence
                            # halves first, then the bf16 h~ matmuls: fewer
                            # DR<->normal weight-pipeline mode switches
                            psF = []
                            psI = []
                            psH = []
                            for half in range(2):
                                s0h = half * 512
                                pf = psA.tile([P, 512], F32, tag="ps",
                                              name="psF")
                                pi = psA.tile([P, 512], F32, tag="ps",
                                              name="psI")
                                psF.append(pf)
                                psI.append(pi)
                                for g, ps in ((0, pf), (1, pi)):
                                    for kp in range(KO // 2):
                                        nc.tensor.matmul(
                                            ps[:],
                                            gws8[g][:, 2 * kp:2 * kp + 2,
                                                    no * P:(no + 1) * P],
                                            X8b_v[b][:, 2 * kp:2 * kp + 2,
                                                     s0h:s0h + 512],
                                            start=(kp == 0),
                                            stop=(kp == KO // 2 - 1),
                                            perf_mode=DR)
                            for half in range(2):
                                m0 = b * S + half * 512
                                ph = psA.tile([P, 512], F32, tag="ps",
                                              name="psH")
                                psH.append(ph)
                                for ko in range(KO):
                                    nc.tensor.matmul(
                                        ph[:],
                                        gwh_t[:, ko * H + no * P:
                                              ko * H + (no + 1) * P],
                                        X[:, ko * BS + m0: ko * BS + m0 + 512],
                                        start=(ko == 0), stop=(ko == KO - 1))
                            bF = gb_sb[:, (l * 3 + 0) * KO + no:
                                       (l * 3 + 0) * KO + no + 1]
                            bI = gb_sb[:, (l * 3 + 1) * KO + no:
                                       (l * 3 + 1) * KO + no + 1]
                            bH = gb_sb[:, (l * 3 + 2) * KO + no:
                                       (l * 3 + 2) * KO + no + 1]
                            for half in range(2):
                                s0 = half * 512
                                d_t = tmpp.tile([P, 512], F32, tag="d_t",
                                                name="d_t")
                                i_t = tmpp.tile([P, 512], DT, tag="i_t",
                                                name="i_t")
                                r_t = tmpp.tile([P, 512], F32, tag="r_t",
                                                name="r_t")
                                w_t = tmpp.tile([P, 512], DT, tag="w_t",
                                                name="w_t")
                                nc.scalar.activation(d_t[:], psF[half][:],
                                                     AFT.Sigmoid, bias=bF,
                                                     scale=RS)
                                nc.scalar.activation(i_t[:], psI[half][:],
                                                     AFT.Sigmoid, bias=bI,
                                                     scale=RS)
                                # d += i on a DMA engine (CCE fp32 add; the
                                # gpsimd queue carries only these after l0)
                                nc.gpsimd.dma_start(d_t[:], i_t[:],
                                                    accum_op=OP.add)
                                nc.vector.reciprocal_approx_fast(r_t[:],
                                                                 d_t[:])
                                nc.vector.tensor_mul(w_t[:], i_t[:], r_t[:])
                                # fp = 1 - w on ACT (affine: w*-1 + 1)
                                nc.scalar.activation(fp_t[:, s0:s0 + 512],
                                                     w_t[:], AFT.Identity,
                                                     bias=1.0, scale=-1.0)
                                nc.vector.scalar_tensor_tensor(
                                    add_t[:, s0:s0 + 512], psH[half][:], bH,
                                    w_t[:], op0=OP.add, op1=OP.mult)
                            # one scan over the batch's whole sequence
                            nc.vector.tensor_tensor_scan(
                                h_out[:, base: base + S],
                                fp_t[:], add_t[:],
                                initial=0.0, op0=OP.mult, op1=OP.add)
                            # fp8 copy for the next layer / attention K,V
                            nc.scalar.activation(
                                h8b_out[b][:, no * S: (no + 1) * S],
                                h_out[:, base: base + S],
                                AFT.Copy, scale=XS)
                    X = h_out
                    X8b = h8b_out

            h4 = X
            h48_v = [t.rearrange("p (k s) -> p k s", k=KO) for t in X8b]

            # ---------------- attention (last query position only) ----------
            with (
                tc.tile_pool(name="vp", bufs=1) as vp,
                tc.tile_pool(name="owp", bufs=1) as owp,
                tc.tile_pool(name="smallp", bufs=1) as smallp,
            ):
                ow_sb = owp.tile([P, KO * H], DT)
                nc.sync.dma_start(ow_sb[:], ow[:])
                fcw_sb = owp.tile([P, KO * O], DT)
                nc.sync.dma_start(fcw_sb[:], fcw[:])
                ipb_sb = constp.tile([P, 2 * KO], F32)
                nc.sync.dma_start(ipb_sb[:], ipb[:])
                vb_sb = constp.tile([P, NH], F32)
                nc.sync.dma_start(vb_sb[:], vb[:])
                ob_sb = constp.tile([P, KO], F32)
                nc.sync.dma_start(ob_sb[:], ob[:])
                fcb_sb = constp.tile([P, O // P], F32)
                nc.sync.dma_start(fcb_sb[:], fcb[:])

                K_sb = hp.tile([P, KO * BS], DT, tag="hbuf", name="K_sb")
                V_sb = vp.tile([P, KO * BS], DT, name="V_sb")
                lastq = smallp.tile([P, 2 * KO], DT)
                lastq8 = smallp.tile([P, 2 * KO], F8D)
                q_sb = smallp.tile([P, 2 * KO], DT)
                e_all = smallp.tile([P, 2 * NH * KO], DT)
                acc_all = smallp.tile([P, 2 * NH], F32)
                den_r = smallp.tile([1, 2 * NH], F32)
                rb_sb = smallp.tile([P, 2 * NH], F32)
                O_last = smallp.tile([P, 2 * KO], DT)
                out_last = smallp.tile([P, 2 * KO], DT)
                res_sb = smallp.tile([P, 2 * (O // P)], F32)

                ip8_v = ip8_sb.rearrange("p (k m) -> p k m", k=KO)

                # h4 columns at the last timestep (per ko-tile, per batch);
                # the fp8 copies feed the Q matmul (gpsimd: other engines
                # are backlogged at the layer/attention boundary)
                for ko in range(KO):
                    for b in range(BC):
                        nc.vector.tensor_copy(
                            lastq[:, ko * BC + b: ko * BC + b + 1],
                            h4[:, ko * BS + b * S + S - 1:
                               ko * BS + b * S + S])
                        nc.vector.tensor_copy(
                            lastq8[:, ko * BC + b: ko * BC + b + 1],
                            X8b[b][:, ko * S + S - 1: ko * S + S])

                def k_chunks(chs):
                    for nt in range(KO):
                        for ch in chs:
                            b, m0 = ch // 2, (ch % 2) * 512
                            ps = psA.tile([P, 512], F32, tag="ps",
                                          name="psk")
                            for kp in range(KO // 2):
                                nc.tensor.matmul(
                                    ps[:],
                                    ip8_v[:, 2 * kp:2 * kp + 2,
                                          H + nt * P: H + (nt + 1) * P],
                                    h48_v[b][:, 2 * kp:2 * kp + 2,
                                             m0:m0 + 512],
                                    start=(kp == 0),
                                    stop=(kp == KO // 2 - 1),
                                    perf_mode=DR)
                            nc.scalar.activation(
                                K_sb[:, nt * BS + ch * 512:
                                     nt * BS + ch * 512 + 512],
                                ps[:], AFT.Identity,
                                bias=ipb_sb[:, KO + nt: KO + nt + 1],
                                scale=RS)

                def v_chunks(sts):
                    for st in sts:
                        b, si = st // KO, st % KO
                        for dch in range(2):
                            d0 = dch * 512
                            ps = psA.tile([P, 512], F32, tag="ps",
                                          name="psv")
                            for kp in range(KO // 2):
                                nc.tensor.matmul(
                                    ps[:],
                                    h48_v[b][:, 2 * kp:2 * kp + 2,
                                             si * P: (si + 1) * P],
                                    ip8_v[:, 2 * kp:2 * kp + 2,
                                          2 * H + d0: 2 * H + d0 + 512],
                                    start=(kp == 0),
                                    stop=(kp == KO // 2 - 1),
                                    perf_mode=DR)
                            nc.scalar.activation(
                                V_sb[:, st * H + d0: st * H + d0 + 512],
                                ps[:], AFT.Identity, scale=RS)

                def o_chunks(bs):
                    for b in bs:
                        for j in range(NH):
                            bj = b * NH + j
                            ps_o = psA.tile([P, 1], F32, tag="sm",
                                            name="ps_o", bufs=2,
                                            padded_shape=[P, 512])
                            for kt in range(KO):
                                nc.tensor.matmul(
                                    ps_o[:],
                                    V_sb[:, (b * KO + kt) * H + j * P:
                                         (b * KO + kt) * H + (j + 1) * P],
                                    e_all[:, bj * KO + kt:
                                          bj * KO + kt + 1],
                                    start=(kt == 0), stop=(kt == KO - 1))
                            nc.vector.scalar_tensor_tensor(
                                O_last[:, j * BC + b: j * BC + b + 1],
                                ps_o[:], rb_sb[:, bj:bj + 1],
                                vb_sb[:, j:j + 1],
                                op0=OP.mult, op1=OP.add)

                # batch-0 K and V run while the layer stack finishes
                # batch 1; then batch-1 K, Q, scores, the softmax-denom
                # chain (hidden under batch-1 V), and the O tail.
                k_chunks((0, 1))
                v_chunks(range(KO))
                k_chunks((2, 3))
                # Q at the last position only (2 columns, fp8 plain)
                for nt in range(KO):
                    ps = psA.tile([P, 512], F32, tag="ps", name="psq")
                    for ko in range(KO):
                        nc.tensor.matmul(
                            ps[:, 0:BC],
                            ip8_v[:, ko, nt * P:(nt + 1) * P],
                            lastq8[:, ko * BC: (ko + 1) * BC],
                            start=(ko == 0), stop=(ko == KO - 1))
                    nc.scalar.activation(
                        q_sb[:, nt * BC: (nt + 1) * BC], ps[:, 0:BC],
                        AFT.Identity, bias=ipb_sb[:, nt: nt + 1],
                        scale=QSCALE * RS)
                # scores + exp (the den chain hides under the V matmuls)
                for b in range(BC):
                    for j in range(NH):
                        bj = b * NH + j
                        ps_s = psA.tile([P, KO], F32, tag="sm",
                                        name="ps_s", bufs=2,
                                        padded_shape=[P, 512])
                        for kt in range(KO):
                            nc.tensor.matmul(
                                ps_s[:, kt:kt + 1],
                                K_sb[:, j * BS + b * S + kt * P:
                                     j * BS + b * S + (kt + 1) * P],
                                q_sb[:, j * BC + b: j * BC + b + 1],
                                start=True, stop=True)
                        nc.scalar.activation(
                            e_all[:, bj * KO: (bj + 1) * KO], ps_s[:],
                            AFT.Exp,
                            accum_out=acc_all[:, bj:bj + 1])
                # denominators: sum acc over partitions -> [1, 16]
                ps_den = psA.tile([1, 2 * NH], F32, tag="sm",
                                  name="ps_den", bufs=2,
                                  padded_shape=[1, 512])
                nc.tensor.matmul(ps_den[:], ones_col[:], acc_all[:],
                                 start=True, stop=True)
                nc.vector.reciprocal(den_r[:], ps_den[:])
                # broadcast reciprocal across partitions -> [128, 16]
                ps_bc = psA.tile([P, 2 * NH], F32, tag="sm", name="ps_bc",
                                 bufs=2, padded_shape=[P, 512])
                nc.tensor.matmul(ps_bc[:], ones_row[:], den_r[:],
                                 start=True, stop=True)
                nc.scalar.activation(rb_sb[:], ps_bc[:], AFT.Copy)
                # batch-1 V (hides the denom chain), interleaved with the
                # O tail so the per-(b,j) DVE work overlaps V matmuls
                v_chunks(range(KO, KO + 4))
                o_chunks((0,))
                v_chunks(range(KO + 4, 2 * KO))
                o_chunks((1,))
                # out projection at last position + residual
                for no in range(KO):
                    ps_p = psA.tile([P, BC], F32, tag="sm", name="ps_p",
                                    bufs=2, padded_shape=[P, 512])
                    for ko in range(KO):
                        nc.tensor.matmul(
                            ps_p[:],
                            ow_sb[:, ko * H + no * P: ko * H + (no + 1) * P],
                            O_last[:, ko * BC: (ko + 1) * BC],
                            start=(ko == 0), stop=(ko == KO - 1))
                    nc.vector.scalar_tensor_tensor(
                        out_last[:, no * BC: (no + 1) * BC],
                        ps_p[:], ob_sb[:, no:no + 1],
                        lastq[:, no * BC: (no + 1) * BC],
                        op0=OP.add, op1=OP.add)
                # final fc
                for ot in range(O // P):
                    ps_f = psA.tile([P, BC], F32, tag="sm", name="ps_f",
                                    bufs=2, padded_shape=[P, 512])
                    for ko in range(KO):
                        nc.tensor.matmul(
                            ps_f[:],
                            fcw_sb[:, ko * O + ot * P: ko * O + (ot + 1) * P],
                            out_last[:, ko * BC: (ko + 1) * BC],
                            start=(ko == 0), stop=(ko == KO - 1))
                    nc.scalar.activation(
                        res_sb[:, ot * BC: (ot + 1) * BC], ps_f[:],
                        AFT.Identity, bias=fcb_sb[:, ot:ot + 1])
                    nc.sync.dma_start(
                        outT[ot * P:(ot + 1) * P, :],
                        res_sb[:, ot * BC: (ot + 1) * BC])

            ip_pool.__exit__(None, None, None)

    nc.compile()
    return nc


def _feature_major(w_t):
    """[H_in, N] (already transposed weight) -> device layout [128, KO*N]."""
    hin, n = w_t.shape
    ko = hin // P
    return np.ascontiguousarray(
        w_t.reshape(ko, P, n).transpose(1, 0, 2).reshape(P, ko * n))


def _prep_inputs(x, Wf, bf, Wi, bi, Wh, bh, in_proj_w, in_proj_b, out_w,
                 out_b, fc_w, fc_b):
    gw8s = []
    gwhs = []
    gbs = []
    for l in range(L):
        for W, bias in ((Wf[l], bf[l]), (Wi[l], bi[l]), (Wh[l], bh[l])):
            gbs.append(bias.reshape(KO, P).T.astype(np.float32))
        gw8s.append(_feature_major(Wf[l].T.astype(np.float32) * WS).astype(F8))
        gw8s.append(_feature_major(Wi[l].T.astype(np.float32) * WS).astype(F8))
        gwhs.append(_feature_major(Wh[l].T.astype(np.float32)).astype(BF16))
    gw8 = np.concatenate(gw8s, axis=0)                   # [8*128, KO*H]
    gwh = np.concatenate(gwhs, axis=0)                   # [4*128, KO*H]
    gb = np.concatenate(gbs, axis=1)                     # [128, 12*KO]
    ipw = in_proj_w.astype(np.float32)
    ip8 = _feature_major(ipw.T * WS).astype(F8)          # [128, KO*3H]
    ipb = in_proj_b[:2 * H].reshape(2 * KO, P).T.astype(np.float32).copy()
    ipb[:, :KO] *= QSCALE                                # fold Q scaling
    vbv = in_proj_b[2 * H:].reshape(NH, P).T.astype(np.float32)
    owp = _feature_major(out_w.T.astype(np.float32)).astype(BF16)
    obv = out_b.reshape(KO, P).T.astype(np.float32)
    fcwp = _feature_major(fc_w.T.astype(np.float32)).astype(BF16)
    fcbv = fc_b.reshape(O // P, P).T.astype(np.float32)
    shared = dict(gw8=gw8, gwh=gwh, gb=np.ascontiguousarray(gb),
                  ip8=ip8, ipb=np.ascontiguousarray(ipb),
                  vb=np.ascontiguousarray(vbv), ow=owp,
                  ob=np.ascontiguousarray(obv), fcw=fcwp,
                  fcb=np.ascontiguousarray(fcbv))
    in_maps = []
    for c in range(NCORES):
        shard = x[c * BC:(c + 1) * BC]                   # [BC, S, H]
        xt = shard.transpose(2, 0, 1).reshape(H, BS)     # [H, BS]
        xt = _feature_major(xt.astype(np.float32))       # [128, KO*BS]
        # fp8 copy in batch-major column order (b, ko, s)
        xt8 = np.ascontiguousarray(
            xt.reshape(P, KO, BC, S).transpose(0, 2, 1, 3)
            .reshape(P, BC * KO * S))
        in_maps.append(dict(shared, xT=xt.astype(BF16),
                            xT8=(xt8 * XS).astype(F8)))
    return in_maps


def kernel(x, Wf, bf, Wi, bi, Wh, bh, in_proj_w, in_proj_b, out_w, out_b,
           fc_w, fc_b):
    from concourse.bass_utils import run_bass_kernel_spmd

    x, Wf, bf, Wi, bi, Wh, bh = (np.asarray(t) for t in
                                 (x, Wf, bf, Wi, bi, Wh, bh))
    in_proj_w, in_proj_b, out_w, out_b, fc_w, fc_b = (
        np.asarray(t) for t in (in_proj_w, in_proj_b, out_w, out_b,
                                fc_w, fc_b))
    if "nc" not in _CACHE:
        _CACHE["nc"] = _build_nc()
    nc = _CACHE["nc"]
    in_maps = _prep_inputs(x, Wf, bf, Wi, bi, Wh, bh, in_proj_w, in_proj_b,
                           out_w, out_b, fc_w, fc_b)
    res = run_bass_kernel_spmd(nc, in_maps, core_ids=list(range(NCORES)))
    _CACHE["last_results"] = res
    out = np.empty((B, O), np.float32)
    for c in range(NCORES):
        outT = res.results[c]["outT"]                    # [O, BC]
        for b in range(BC):
            out[c * BC + b] = outT[:, b]
    return out
